# revision 8
# baseline (speedup 1.0000x reference)
"""Trainium2 Bass kernel for nn_BERTClassifier (batch-mixing attention BERT).

The reference output depends only on sequence position 0 (attention mixes the
batch within a position; every other op is position-local), so the real work
is a [32, 768] activation through 4 transformer layers — dominated by weight
streaming and inter-core latency, not FLOPs.

Design (v2):
- Everything is FEATURE-major ([128 partitions, 6 chunks, 32 tokens]): vector
  ops use all 128 lanes (4x faster than token-major), and every projection is
  a run of N=32 matmuls (lhsT = weight chunk, rhs = activation), which the PE
  pipelines at ~27ns/mm — the N=384 weight-moving form serializes on
  LDWEIGHTS at ~320ns/mm.
- Attention algebra folded on the host: scores = x (Wq Wk^T/sqrt(E)) x^T and
  ao Wo = (attn x) (Wv Wo). So the device sees TWO 768x768 matrices per layer
  (Wqk, Wvo) instead of four — half the attention HBM traffic and no separate
  Q/K/V/ao stages. Attention weights replicated on all 8 cores; FFN is 8-way
  tensor parallel (W1 col-shard, W2 row-shard) with ONE partial-sum exchange
  per layer.
- Exchange = all-to-all remote_dma_broadcast (SBUF->SBUF, XOR-delta slotting:
  delta d lands in recv slot d of core me^d), gated by a remote semaphore.
  No ncfw collectives at all -> no CC-stream entry barrier (37-85us in the
  baseline), no DRAM staging round trips (~15us/collective). The arrival
  waits are attached AFTER Tile scheduling (the scheduler's single-core sim
  cannot see remote increments and would deadlock).
- LayerNorm feature-major: PE ones-matmul column sums, tiny [1,32] stat ops,
  PE broadcast, fused normalize. Softmax skips max-subtraction (scores are
  O(1) by construction) and folds the 1/sum into the attn@x PSUM copy.
- Weights are cast fp32->fp16 on the HOST (halves HBM traffic; HWDGE loads).

Biases/affine are all zero in this problem instance; if a caller ever passes
nonzero ones, kernel() falls back to an exact numpy implementation.

Self-contained: shapes hardcoded, no sibling imports.
"""
import sys
import types

import numpy as np

# If BASS_TRACE is set but the axon NTFF hook module is absent, bass_utils
# would crash importing antenv.axon_hooks. Provide a null hook so tracing
# degrades to a warning instead. (test.py installs the real hook first.)
try:
    from antenv import axon_hooks as _ah  # noqa: F401
except ImportError:
    try:
        import antenv as _antenv
        _mod = types.ModuleType("antenv.axon_hooks")
        _mod.get_axon_ntff_profile_hook = lambda: None
        _mod.set_axon_ntff_profile_hook = lambda h: None
        _antenv.axon_hooks = _mod
        sys.modules["antenv.axon_hooks"] = _mod
    except Exception:
        pass

import concourse.bass as bass
import concourse.bacc as bacc
import concourse.mybir as mybir
import concourse.tile as tile
from concourse import masks
from concourse.bass_utils import run_bass_kernel_spmd

F32 = mybir.dt.float32
F16 = mybir.dt.float16
AX = mybir.AxisListType
ALU = mybir.AluOpType
ACT_F = mybir.ActivationFunctionType

V, E, F, L, S, B, C = 30522, 768, 3072, 4, 512, 32, 2
NC = 8             # cores
FSH = F // NC      # 384 ffn shard
KC = E // 128      # 6 contraction chunks of 128
KF = FSH // 128    # 3 contraction chunks for the W2 shard
NH = E // 2        # 384
SCALE = 1.0 / float(np.sqrt(E))
EPS = 1e-5

USE_REMOTE = True  # remote_dma exchange vs ncfw AllReduce

_CACHE = {}
LAST_RESULT = None  # BassKernelResults of the most recent run (for test.py)


def _declare(nc):
    h = {}
    h["x0T"] = nc.dram_tensor("x0T", [E, B], F32, kind="ExternalInput")
    h["x0tok"] = nc.dram_tensor("x0tok", [B, E], F16, kind="ExternalInput")
    for l in range(L):
        h[f"wqk{l}"] = nc.dram_tensor(f"wqk{l}", [E, E], F16, kind="ExternalInput")
        h[f"wvo{l}"] = nc.dram_tensor(f"wvo{l}", [E, E], F16, kind="ExternalInput")
        h[f"w1{l}"] = nc.dram_tensor(f"w1{l}", [E, FSH], F16, kind="ExternalInput")
        h[f"w2{l}"] = nc.dram_tensor(f"w2{l}", [FSH, E], F16, kind="ExternalInput")
    h["wc"] = nc.dram_tensor("wc", [E, C], F16, kind="ExternalInput")
    h["out"] = nc.dram_tensor("out", [B, C], F32, kind="ExternalOutput")
    return h


def _emit(tc, h):
    nc = tc.nc
    groups = [list(range(NC))]
    ctxs = []
    post_waits = []  # (BassInstruction, sem, threshold) applied post-scheduling

    def pool(*a, **k):
        p = tc.alloc_tile_pool(*a, **k)
        ctxs.append(p)
        return p

    const = pool(name="const", bufs=1)
    wp = pool(name="wts", bufs=2)
    ab = pool(name="act", bufs=2)
    ps = pool(name="ps", bufs=2, space="PSUM")

    ident32 = const.tile([B, B], F16)
    masks.make_identity(nc, ident32[:])
    ident128 = const.tile([128, 128], F16)
    masks.make_identity(nc, ident128[:])
    ones_col = const.tile([128, 1], F32)       # scaled by 1/E: mm gives mean
    nc.vector.memset(ones_col[:], 1.0 / E)
    ones_row = const.tile([1, 128], F32)
    nc.vector.memset(ones_row[:], 1.0)
    eps_sb = const.tile([1, 1], F32)
    nc.vector.memset(eps_sb[:], EPS)

    if USE_REMOTE:
        rsem = nc.alloc_semaphore("rsem")
        lsem = nc.alloc_semaphore("lsem")
        # per-layer slots, never reused: no WAR hazards across layers.
        sendb = const.tile([128, L, 192], F16)
        recvb = const.tile([128, L, 9, 192], F16)
    else:
        dr = pool(name="dram", bufs=2, space="DRAM")

    # ---- embedding (position 0 only), both layouts from host
    xT = ab.tile([128, KC, B], F32, tag="xT0")
    nc.scalar.dma_start(xT[:], h["x0T"].ap().rearrange("(k p) b -> p k b", p=128))
    xT16 = ab.tile([128, KC, B], F16, tag="xT16_0")
    nc.vector.tensor_copy(xT16[:], xT[:])
    x_tok16 = ab.tile([B, E], F16, tag="xtok0")
    nc.scalar.dma_start(x_tok16[:], h["x0tok"].ap())

    def load_w(name, rows, cols):
        t = wp.tile([128, rows // 128, cols], F16, tag=name[:3])
        nc.sync.dma_start(t[:], h[name].ap().rearrange("(k p) n -> p k n", p=128))
        return t

    def layernorm(y_sb, out_tags):
        # feature-major LN: y_sb [128, KC, B] f32 -> (xnT f32, xnT16 f16)
        sq = ab.tile([128, KC, B], F32, tag="sq")
        nc.vector.tensor_tensor(sq[:], y_sb[:], y_sb[:], op=ALU.mult)
        mu_ps = ps.tile([1, KC, B], F32, tag="st")
        nc.tensor.matmul(mu_ps[:], ones_col[:], y_sb[:], start=True, stop=True)
        m2_ps = ps.tile([1, KC, B], F32, tag="st")
        nc.tensor.matmul(m2_ps[:], ones_col[:], sq[:], start=True, stop=True)
        stat = ab.tile([1, 2, B], F32, tag="stat")
        nc.vector.tensor_reduce(
            stat[:, 0, :], mu_ps[:].rearrange("p k b -> p b k"), axis=AX.X, op=ALU.add)
        nc.vector.tensor_reduce(
            stat[:, 1, :], m2_ps[:].rearrange("p k b -> p b k"), axis=AX.X, op=ALU.add)
        musq = ab.tile([1, B], F32, tag="musq")
        nc.vector.tensor_tensor(musq[:], stat[:, 0, :], stat[:, 0, :], op=ALU.mult)
        var = ab.tile([1, B], F32, tag="var")
        nc.vector.tensor_tensor(var[:], stat[:, 1, :], musq[:], op=ALU.subtract)
        sd = ab.tile([1, B], F32, tag="sd")
        nc.scalar.activation(sd[:], var[:], ACT_F.Sqrt, bias=eps_sb[:])
        nc.vector.reciprocal(stat[:, 1, :], sd[:])
        bc_ps = ps.tile([128, 2, B], F32, tag="st")
        nc.tensor.matmul(bc_ps[:], ones_row[:], stat[:], start=True, stop=True)
        mu_bb = bc_ps[:, 0:1, :].broadcast_to([128, KC, B])
        rs_bb = bc_ps[:, 1:2, :].broadcast_to([128, KC, B])
        tmp = ab.tile([128, KC, B], F32, tag="lntmp")
        nc.vector.tensor_tensor(tmp[:], y_sb[:], mu_bb, op=ALU.subtract)
        xnT = ab.tile([128, KC, B], F32, tag=out_tags[0])
        nc.vector.tensor_tensor(xnT[:], tmp[:], rs_bb, op=ALU.mult)
        xnT16 = ab.tile([128, KC, B], F16, tag=out_tags[1])
        nc.vector.tensor_copy(xnT16[:], xnT[:])
        return xnT, xnT16

    for l in range(L):
        wqk = load_w(f"wqk{l}", E, E)
        wvo = load_w(f"wvo{l}", E, E)
        w1 = load_w(f"w1{l}", E, FSH)
        w2 = load_w(f"w2{l}", FSH, E)

        # --- t^T = (x Wqk)^T  [128, KC, 32]
        tT_ps = ps.tile([128, KC, B], F32, tag="fm")
        for m in range(KC):
            for k in range(KC):
                nc.tensor.matmul(tT_ps[:, m, :], wqk[:, k, 128 * m:128 * (m + 1)],
                                 xT16[:, k, :], start=(k == 0), stop=(k == KC - 1))
        tT16 = ab.tile([128, KC, B], F16, tag="tT16")
        nc.vector.tensor_copy(tT16[:], tT_ps[:])

        # --- scores = t @ x^T [32, 32]; softmax without max-subtraction
        sc_ps = ps.tile([B, B], F32, tag="sm", bufs=1)
        for k in range(KC):
            nc.tensor.matmul(sc_ps[:], tT16[:, k, :], xT16[:, k, :],
                             start=(k == 0), stop=(k == KC - 1))
        attn16 = ab.tile([B, B], F16, tag="attn16")
        rsum = ab.tile([B, 1], F32, tag="rsum")
        nc.scalar.activation(attn16[:], sc_ps[:], ACT_F.Exp, accum_out=rsum[:])
        rinv = ab.tile([B, 1], F32, tag="rinv")
        nc.vector.reciprocal(rinv[:], rsum[:])
        attnT16 = ab.tile([B, B], F16, tag="attnT16")
        nc.vector.transpose(attnT16[:], attn16[:])

        # --- xa = softmax(scores) @ x  (token mixing; 1/sum folded into copy)
        xa_ps0 = ps.tile([B, NH], F32, tag="tm")
        xa_ps1 = ps.tile([B, NH], F32, tag="tm")
        for n, xps in enumerate((xa_ps0, xa_ps1)):
            nc.tensor.matmul(xps[:], attnT16[:], x_tok16[:, NH * n:NH * (n + 1)],
                             start=True, stop=True)
        xa16 = ab.tile([B, E], F16, tag="xa16")
        nc.scalar.activation(xa16[:, 0:NH], xa_ps0[:], ACT_F.Copy, scale=rinv[:])
        nc.scalar.activation(xa16[:, NH:E], xa_ps1[:], ACT_F.Copy, scale=rinv[:])

        # --- xa^T via PE transposes, then o^T = (xa Wvo)^T
        xaT_ps = ps.tile([128, KC, B], F16, tag="fmT", bufs=1)
        for j in range(KC):
            nc.tensor.transpose(xaT_ps[:, j, :], xa16[:, 128 * j:128 * (j + 1)],
                                ident32[:])
        xaT16 = ab.tile([128, KC, B], F16, tag="xaT16")
        nc.vector.tensor_copy(xaT16[:], xaT_ps[:])
        oT_ps = ps.tile([128, KC, B], F32, tag="fm")
        for m in range(KC):
            for k in range(KC):
                nc.tensor.matmul(oT_ps[:, m, :], wvo[:, k, 128 * m:128 * (m + 1)],
                                 xaT16[:, k, :], start=(k == 0), stop=(k == KC - 1))

        # --- residual + LN1
        y1 = ab.tile([128, KC, B], F32, tag="y1")
        nc.vector.tensor_tensor(y1[:], xT[:], oT_ps[:], op=ALU.add)
        x1nT, x1nT16 = layernorm(y1, ("x1nT", "x1nT16"))

        # --- FFN1 shard: h^T = relu(W1_c^T x1n^T) [128, KF, 32]
        hT_ps = ps.tile([128, KF, B], F32, tag="fm")
        for m in range(KF):
            for k in range(KC):
                nc.tensor.matmul(hT_ps[:, m, :], w1[:, k, 128 * m:128 * (m + 1)],
                                 x1nT16[:, k, :], start=(k == 0), stop=(k == KC - 1))
        hT16 = ab.tile([128, KF, B], F16, tag="hT16")
        nc.vector.tensor_scalar_max(hT16[:], hT_ps[:], 0.0)

        # --- FFN2 shard partial: o2p^T = W2_c^T h^T [128, KC, 32]
        o2_ps = ps.tile([128, KC, B], F32, tag="fm")
        for m in range(KC):
            for k in range(KF):
                nc.tensor.matmul(o2_ps[:, m, :], w2[:, k, 128 * m:128 * (m + 1)],
                                 hT16[:, k, :], start=(k == 0), stop=(k == KF - 1))

        # --- exchange: sum the 8 partial o2^T across cores
        if USE_REMOTE:
            send = sendb[:, l, :].rearrange("p (k b) -> p k b", k=KC)
            nc.vector.tensor_copy(send, o2_ps[:])
            nc.vector.tensor_copy(
                recvb[:, l, 0, :].rearrange("p (k b) -> p k b", k=KC), o2_ps[:])
            for d in range(1, 8):
                rd = [None] * 8
                rd[d] = (0, d)
                nc.gpsimd.remote_dma_broadcast(
                    recvb[:, l, d, :], sendb[:, l, :],
                    remote_sem=rsem, local_sem=lsem, rdests=rd)
            nc.gpsimd.trigger_dma(count=None)
            # gate writes zeros to slot 8 AND reads the send tile: the read
            # forces Tile to schedule it after the send-copy, so every core's
            # sends are in flight before its vector queue blocks on arrivals
            # (a dep-free memset could legally be scheduled first -> global
            # deadlock). The rsem wait itself is attached post-scheduling.
            gate = nc.vector.tensor_scalar_mul(recvb[:, l, 8, :], sendb[:, l, :], 0.0)
            post_waits.append((gate, rsem, 14 * (l + 1)))
            o2s = ab.tile([128, KC, B], F32, tag="o2s")
            nc.vector.tensor_reduce(
                o2s[:], recvb[:, l, :, :].rearrange("p s (k b) -> p k b s", k=KC),
                axis=AX.X, op=ALU.add)
        else:
            o2p = ab.tile([128, KC, B], F32, tag="o2p")
            nc.vector.tensor_copy(o2p[:], o2_ps[:])
            ar_i = dr.tile([128, KC * B], F32, tag="ari")
            ar_o = dr.tile([128, KC * B], F32, addr_space="Shared", tag="aro")
            nc.scalar.dma_start(ar_i[:].rearrange("p (k b) -> p k b", k=KC), o2p[:])
            nc.gpsimd.collective_compute(
                "AllReduce", ALU.add, replica_groups=groups,
                ins=[ar_i.opt()], outs=[ar_o.opt()],
            )
            o2s = ab.tile([128, KC, B], F32, tag="o2s")
            nc.scalar.dma_start(o2s[:], ar_o[:].rearrange("p (k b) -> p k b", k=KC))

        # --- residual + LN2
        y2 = ab.tile([128, KC, B], F32, tag="y2")
        nc.vector.tensor_tensor(y2[:], x1nT[:], o2s[:], op=ALU.add)
        xT, xT16 = layernorm(y2, ("xT", "xT16"))

        # --- token-major x for the NEXT layer's attention mix
        if l < L - 1:
            xtok_ps0 = ps.tile([B, NH], F16, tag="tm")
            xtok_ps1 = ps.tile([B, NH], F16, tag="tm")
            for j in range(KC):
                tgt = xtok_ps0 if j < 3 else xtok_ps1
                nc.tensor.transpose(tgt[:, 128 * (j % 3):128 * (j % 3 + 1)],
                                    xT16[:, j, :], ident128[:])
            x_tok16 = ab.tile([B, E], F16, tag="xtok")
            nc.scalar.activation(x_tok16[:, 0:NH], xtok_ps0[:], ACT_F.Copy)
            nc.scalar.activation(x_tok16[:, NH:E], xtok_ps1[:], ACT_F.Copy)

    # --- classifier: logits = x @ Wc
    wc_sb = wp.tile([128, KC, C], F16, tag="wcs")
    nc.sync.dma_start(wc_sb[:], h["wc"].ap().rearrange("(k p) n -> p k n", p=128))
    lg_ps = ps.tile([B, C], F32, tag="sm", bufs=1)
    for k in range(KC):
        nc.tensor.matmul(lg_ps[:], xT16[:, k, :], wc_sb[:, k, :],
                         start=(k == 0), stop=(k == KC - 1))
    lg_sb = ab.tile([B, C], F32, tag="lgs")
    nc.vector.tensor_copy(lg_sb[:], lg_ps[:])
    nc.sync.dma_start(h["out"].ap(), lg_sb[:])

    for p in reversed(ctxs):
        p.release()
    return post_waits


def build():
    if "k" in _CACHE:
        return _CACHE["k"]
    nc = bacc.Bacc("TRN2", target_bir_lowering=False, debug=False, num_devices=NC)
    h = _declare(nc)
    with tile.TileContext(nc) as tc:
        post_waits = _emit(tc, h)
    for inst, sem, thr in post_waits:
        inst._wait_ge(sem, thr)
    nc.compile()
    _CACHE["k"] = (nc, h)
    return nc, h


def make_in_maps(inputs):
    f32 = lambda a: np.asarray(a, dtype=np.float32)
    ids = np.asarray(inputs["input_ids"])[0]
    x0 = f32(inputs["tok_emb"])[ids] + f32(inputs["pos_emb"])[0][None, :]
    x0T = np.ascontiguousarray(x0.T.astype(np.float32))      # [768, 32] f32
    x0tok = np.ascontiguousarray(x0.astype(np.float16))      # [32, 768] f16

    shared = {"x0T": x0T, "x0tok": x0tok,
              "wc": np.ascontiguousarray(f32(inputs["Wc"]).astype(np.float16))}
    wqk, wvo, w1, w2 = [], [], [], []
    for l in range(L):
        qk = (f32(inputs["Wq"][l]) @ f32(inputs["Wk"][l]).T) * SCALE
        vo = f32(inputs["Wv"][l]) @ f32(inputs["Wo"][l])
        wqk.append(np.ascontiguousarray(qk.astype(np.float16)))
        wvo.append(np.ascontiguousarray(vo.astype(np.float16)))
        w1.append(f32(inputs["W1"][l]).astype(np.float16))
        w2.append(f32(inputs["W2"][l]).astype(np.float16))

    in_maps = []
    for c in range(NC):
        m = dict(shared)
        for l in range(L):
            m[f"wqk{l}"] = wqk[l]
            m[f"wvo{l}"] = wvo[l]
            m[f"w1{l}"] = np.ascontiguousarray(w1[l][:, FSH * c:FSH * (c + 1)])
            m[f"w2{l}"] = np.ascontiguousarray(w2[l][FSH * c:FSH * (c + 1), :])
        in_maps.append(m)
    return in_maps


def _nontrivial_bias(inputs):
    z = lambda *names: all(not np.any(np.asarray(inputs[n])) for n in names)
    use_bias = not z("bq", "bk", "bv", "bo", "bf1", "bf2", "bc")
    use_affine = not (
        z("beta1", "beta2")
        and np.all(np.asarray(inputs["g1"]) == 1.0)
        and np.all(np.asarray(inputs["g2"]) == 1.0)
    )
    return use_bias or use_affine


def _numpy_reference(inputs):
    # Exact CPU fallback (only taken if biases/affine are nontrivial).
    I = {k: np.asarray(v) for k, v in inputs.items()}
    x = I["tok_emb"][I["input_ids"]] + I["pos_emb"][np.arange(S)][:, None, :]
    x = x.astype(np.float32)
    scale = 1.0 / np.sqrt(E)

    def ln(t, g, b):
        mu = t.mean(-1, keepdims=True)
        var = t.var(-1, keepdims=True)
        return (t - mu) / np.sqrt(var + 1e-5) * g + b

    for l in range(L):
        Q = x @ I["Wq"][l] + I["bq"][l]
        K = x @ I["Wk"][l] + I["bk"][l]
        Vv = x @ I["Wv"][l] + I["bv"][l]
        sc = np.einsum('sbe,sce->sbc', Q, K) * scale
        sc = sc - sc.max(-1, keepdims=True)
        a = np.exp(sc)
        a /= a.sum(-1, keepdims=True)
        ao = np.einsum('sbc,sce->sbe', a, Vv) @ I["Wo"][l] + I["bo"][l]
        x = ln(x + ao, I["g1"][l], I["beta1"][l])
        hh = np.maximum(x @ I["W1"][l] + I["bf1"][l], 0.0) @ I["W2"][l] + I["bf2"][l]
        x = ln(x + hh, I["g2"][l], I["beta2"][l])
    return (x[0] @ I["Wc"] + I["bc"]).astype(np.float32)


def kernel(**inputs) -> np.ndarray:
    global LAST_RESULT
    if _nontrivial_bias(inputs):
        return _numpy_reference(inputs)
    nc, h = build()
    in_maps = make_in_maps(inputs)
    res = run_bass_kernel_spmd(nc, in_maps, core_ids=list(range(NC)))
    LAST_RESULT = res
    return np.asarray(res.results[0]["out"])


# revision 9
# speedup vs baseline: 24.3865x; 24.3865x over previous
"""Trainium2 Bass kernel for nn_BERTClassifier (batch-mixing attention BERT).

The reference output depends only on sequence position 0 (attention mixes the
batch within a position; every other op is position-local), so the real work
is a [32, 768] activation through 4 transformer layers — dominated by weight
streaming and inter-core latency, not FLOPs.

Design (v2):
- Everything is FEATURE-major ([128 partitions, 6 chunks, 32 tokens]): vector
  ops use all 128 lanes (4x faster than token-major), and every projection is
  a run of N=32 matmuls (lhsT = weight chunk, rhs = activation), which the PE
  pipelines at ~27ns/mm — the N=384 weight-moving form serializes on
  LDWEIGHTS at ~320ns/mm.
- Attention algebra folded on the host: scores = x (Wq Wk^T/sqrt(E)) x^T and
  ao Wo = (attn x) (Wv Wo). So the device sees TWO 768x768 matrices per layer
  (Wqk, Wvo) instead of four — half the attention HBM traffic and no separate
  Q/K/V/ao stages. Attention weights replicated on all 8 cores; FFN is 8-way
  tensor parallel (W1 col-shard, W2 row-shard) with ONE partial-sum exchange
  per layer.
- Exchange = all-to-all remote_dma_broadcast (SBUF->SBUF, XOR-delta slotting:
  delta d lands in recv slot d of core me^d), gated by a remote semaphore.
  No ncfw collectives at all -> no CC-stream entry barrier (37-85us in the
  baseline), no DRAM staging round trips (~15us/collective). The arrival
  waits are attached AFTER Tile scheduling (the scheduler's single-core sim
  cannot see remote increments and would deadlock).
- LayerNorm feature-major: PE ones-matmul column sums, tiny [1,32] stat ops,
  PE broadcast, fused normalize. Softmax skips max-subtraction (scores are
  O(1) by construction) and folds the 1/sum into the attn@x PSUM copy.
- Weights are cast fp32->fp16 on the HOST (halves HBM traffic; HWDGE loads).

Biases/affine are all zero in this problem instance; if a caller ever passes
nonzero ones, kernel() falls back to an exact numpy implementation.

Self-contained: shapes hardcoded, no sibling imports.
"""
import sys
import types

import numpy as np

# If BASS_TRACE is set but the axon NTFF hook module is absent, bass_utils
# would crash importing antenv.axon_hooks. Provide a null hook so tracing
# degrades to a warning instead. (test.py installs the real hook first.)
try:
    from antenv import axon_hooks as _ah  # noqa: F401
except ImportError:
    try:
        import antenv as _antenv
        _mod = types.ModuleType("antenv.axon_hooks")
        _mod.get_axon_ntff_profile_hook = lambda: None
        _mod.set_axon_ntff_profile_hook = lambda h: None
        _antenv.axon_hooks = _mod
        sys.modules["antenv.axon_hooks"] = _mod
    except Exception:
        pass

import concourse.bass as bass
import concourse.bacc as bacc
import concourse.mybir as mybir
import concourse.tile as tile
from concourse import masks
from concourse.bass_utils import run_bass_kernel_spmd

F32 = mybir.dt.float32
F16 = mybir.dt.float16
AX = mybir.AxisListType
ALU = mybir.AluOpType
ACT_F = mybir.ActivationFunctionType

V, E, F, L, S, B, C = 30522, 768, 3072, 4, 512, 32, 2
NC = 8             # cores
FSH = F // NC      # 384 ffn shard
KC = E // 128      # 6 contraction chunks of 128
KF = FSH // 128    # 3 contraction chunks for the W2 shard
NH = E // 2        # 384
SCALE = 1.0 / float(np.sqrt(E))
EPS = 1e-5

USE_REMOTE = False  # remote_dma exchange vs ncfw AllReduce (remote_dma
# delivery measured multi-ms in this axon-tunneled environment; ncfw
# collectives deliver in ~10us, so the exchange rides ncfw)

_CACHE = {}
LAST_RESULT = None  # BassKernelResults of the most recent run (for test.py)


def _declare(nc):
    h = {}
    h["x0T"] = nc.dram_tensor("x0T", [E, B], F32, kind="ExternalInput")
    h["x0tok"] = nc.dram_tensor("x0tok", [B, E], F16, kind="ExternalInput")
    for l in range(L):
        h[f"wqk{l}"] = nc.dram_tensor(f"wqk{l}", [E, E], F16, kind="ExternalInput")
        h[f"wvo{l}"] = nc.dram_tensor(f"wvo{l}", [E, E], F16, kind="ExternalInput")
        h[f"w1{l}"] = nc.dram_tensor(f"w1{l}", [E, FSH], F16, kind="ExternalInput")
        h[f"w2{l}"] = nc.dram_tensor(f"w2{l}", [FSH, E], F16, kind="ExternalInput")
    h["wc"] = nc.dram_tensor("wc", [E, C], F16, kind="ExternalInput")
    h["out"] = nc.dram_tensor("out", [B, C], F32, kind="ExternalOutput")
    return h


def _emit(tc, h):
    nc = tc.nc
    groups = [list(range(NC))]
    ctxs = []
    post_waits = []  # (BassInstruction, sem, threshold) applied post-scheduling

    def pool(*a, **k):
        p = tc.alloc_tile_pool(*a, **k)
        ctxs.append(p)
        return p

    const = pool(name="const", bufs=1)
    wp = pool(name="wts", bufs=2)
    ab = pool(name="act", bufs=2)
    ps = pool(name="ps", bufs=2, space="PSUM")

    ident32 = const.tile([B, B], F16)
    masks.make_identity(nc, ident32[:])
    ident128 = const.tile([128, 128], F16)
    masks.make_identity(nc, ident128[:])
    ones_col = const.tile([128, 1], F32)       # scaled by 1/E: mm gives mean
    nc.vector.memset(ones_col[:], 1.0 / E)
    ones_row = const.tile([1, 128], F32)
    nc.vector.memset(ones_row[:], 1.0)
    eps_sb = const.tile([1, 1], F32)
    nc.vector.memset(eps_sb[:], EPS)

    if USE_REMOTE:
        rsem = nc.alloc_semaphore("rsem")
        lsem = nc.alloc_semaphore("lsem")
        # per-layer slots, never reused: no WAR hazards across layers.
        sendb = const.tile([128, L, 192], F16)
        recvb = const.tile([128, L, 9, 192], F16)
    else:
        dr = pool(name="dram", bufs=2, space="DRAM")

    # ---- embedding (position 0 only), both layouts from host
    xT = ab.tile([128, KC, B], F32, tag="xT0")
    nc.scalar.dma_start(xT[:], h["x0T"].ap().rearrange("(k p) b -> p k b", p=128))
    xT16 = ab.tile([128, KC, B], F16, tag="xT16_0")
    nc.vector.tensor_copy(xT16[:], xT[:])
    x_tok16 = ab.tile([B, E], F16, tag="xtok0")
    nc.scalar.dma_start(x_tok16[:], h["x0tok"].ap())

    def load_w(name, rows, cols):
        t = wp.tile([128, rows // 128, cols], F16, tag=name[:3])
        nc.sync.dma_start(t[:], h[name].ap().rearrange("(k p) n -> p k n", p=128))
        return t

    def layernorm(y_sb, out_tags):
        # feature-major LN: y_sb [128, KC, B] f32 -> (xnT f32, xnT16 f16)
        sq = ab.tile([128, KC, B], F32, tag="sq")
        nc.vector.tensor_tensor(sq[:], y_sb[:], y_sb[:], op=ALU.mult)
        mu_ps = ps.tile([1, KC, B], F32, tag="st")
        nc.tensor.matmul(mu_ps[:], ones_col[:], y_sb[:], start=True, stop=True)
        m2_ps = ps.tile([1, KC, B], F32, tag="st")
        nc.tensor.matmul(m2_ps[:], ones_col[:], sq[:], start=True, stop=True)
        stat = ab.tile([1, 2, B], F32, tag="stat")
        nc.vector.tensor_reduce(
            stat[:, 0, :], mu_ps[:].rearrange("p k b -> p b k"), axis=AX.X, op=ALU.add)
        nc.vector.tensor_reduce(
            stat[:, 1, :], m2_ps[:].rearrange("p k b -> p b k"), axis=AX.X, op=ALU.add)
        musq = ab.tile([1, B], F32, tag="musq")
        nc.vector.tensor_tensor(musq[:], stat[:, 0, :], stat[:, 0, :], op=ALU.mult)
        var = ab.tile([1, B], F32, tag="var")
        nc.vector.tensor_tensor(var[:], stat[:, 1, :], musq[:], op=ALU.subtract)
        sd = ab.tile([1, B], F32, tag="sd")
        nc.scalar.activation(sd[:], var[:], ACT_F.Sqrt, bias=eps_sb[:])
        nc.vector.reciprocal(stat[:, 1, :], sd[:])
        bc_ps = ps.tile([128, 2, B], F32, tag="st")
        nc.tensor.matmul(bc_ps[:], ones_row[:], stat[:], start=True, stop=True)
        mu_bb = bc_ps[:, 0:1, :].broadcast_to([128, KC, B])
        rs_bb = bc_ps[:, 1:2, :].broadcast_to([128, KC, B])
        tmp = ab.tile([128, KC, B], F32, tag="lntmp")
        nc.vector.tensor_tensor(tmp[:], y_sb[:], mu_bb, op=ALU.subtract)
        xnT = ab.tile([128, KC, B], F32, tag=out_tags[0])
        nc.vector.tensor_tensor(xnT[:], tmp[:], rs_bb, op=ALU.mult)
        xnT16 = ab.tile([128, KC, B], F16, tag=out_tags[1])
        nc.vector.tensor_copy(xnT16[:], xnT[:])
        return xnT, xnT16

    for l in range(L):
        wqk = load_w(f"wqk{l}", E, E)
        wvo = load_w(f"wvo{l}", E, E)
        w1 = load_w(f"w1{l}", E, FSH)
        w2 = load_w(f"w2{l}", FSH, E)

        # --- t^T = (x Wqk)^T  [128, KC, 32]
        tT_ps = ps.tile([128, KC, B], F32, tag="fm")
        for m in range(KC):
            for k in range(KC):
                nc.tensor.matmul(tT_ps[:, m, :], wqk[:, k, 128 * m:128 * (m + 1)],
                                 xT16[:, k, :], start=(k == 0), stop=(k == KC - 1))
        tT16 = ab.tile([128, KC, B], F16, tag="tT16")
        nc.vector.tensor_copy(tT16[:], tT_ps[:])

        # --- scores = t @ x^T [32, 32]; softmax without max-subtraction
        sc_ps = ps.tile([B, B], F32, tag="sm", bufs=1)
        for k in range(KC):
            nc.tensor.matmul(sc_ps[:], tT16[:, k, :], xT16[:, k, :],
                             start=(k == 0), stop=(k == KC - 1))
        attn16 = ab.tile([B, B], F16, tag="attn16")
        rsum = ab.tile([B, 1], F32, tag="rsum")
        nc.scalar.activation(attn16[:], sc_ps[:], ACT_F.Exp, accum_out=rsum[:])
        rinv = ab.tile([B, 1], F32, tag="rinv")
        nc.vector.reciprocal(rinv[:], rsum[:])
        attnT16 = ab.tile([B, B], F16, tag="attnT16")
        nc.vector.transpose(attnT16[:], attn16[:])

        # --- xa = softmax(scores) @ x  (token mixing; 1/sum folded into copy)
        xa_ps0 = ps.tile([B, NH], F32, tag="tm")
        xa_ps1 = ps.tile([B, NH], F32, tag="tm")
        for n, xps in enumerate((xa_ps0, xa_ps1)):
            nc.tensor.matmul(xps[:], attnT16[:], x_tok16[:, NH * n:NH * (n + 1)],
                             start=True, stop=True)
        xa16 = ab.tile([B, E], F16, tag="xa16")
        nc.scalar.activation(xa16[:, 0:NH], xa_ps0[:], ACT_F.Copy, scale=rinv[:])
        nc.scalar.activation(xa16[:, NH:E], xa_ps1[:], ACT_F.Copy, scale=rinv[:])

        # --- xa^T via PE transposes, then o^T = (xa Wvo)^T
        xaT_ps = ps.tile([128, KC, B], F16, tag="fmT", bufs=1)
        for j in range(KC):
            nc.tensor.transpose(xaT_ps[:, j, :], xa16[:, 128 * j:128 * (j + 1)],
                                ident32[:])
        xaT16 = ab.tile([128, KC, B], F16, tag="xaT16")
        nc.vector.tensor_copy(xaT16[:], xaT_ps[:])
        oT_ps = ps.tile([128, KC, B], F32, tag="fm")
        for m in range(KC):
            for k in range(KC):
                nc.tensor.matmul(oT_ps[:, m, :], wvo[:, k, 128 * m:128 * (m + 1)],
                                 xaT16[:, k, :], start=(k == 0), stop=(k == KC - 1))

        # --- residual + LN1
        y1 = ab.tile([128, KC, B], F32, tag="y1")
        nc.vector.tensor_tensor(y1[:], xT[:], oT_ps[:], op=ALU.add)
        x1nT, x1nT16 = layernorm(y1, ("x1nT", "x1nT16"))

        # --- FFN1 shard: h^T = relu(W1_c^T x1n^T) [128, KF, 32]
        hT_ps = ps.tile([128, KF, B], F32, tag="fm")
        for m in range(KF):
            for k in range(KC):
                nc.tensor.matmul(hT_ps[:, m, :], w1[:, k, 128 * m:128 * (m + 1)],
                                 x1nT16[:, k, :], start=(k == 0), stop=(k == KC - 1))
        hT16 = ab.tile([128, KF, B], F16, tag="hT16")
        nc.vector.tensor_scalar_max(hT16[:], hT_ps[:], 0.0)

        # --- FFN2 shard partial: o2p^T = W2_c^T h^T [128, KC, 32]
        o2_ps = ps.tile([128, KC, B], F32, tag="fm")
        for m in range(KC):
            for k in range(KF):
                nc.tensor.matmul(o2_ps[:, m, :], w2[:, k, 128 * m:128 * (m + 1)],
                                 hT16[:, k, :], start=(k == 0), stop=(k == KF - 1))

        # --- exchange: sum the 8 partial o2^T across cores
        if USE_REMOTE:
            send = sendb[:, l, :].rearrange("p (k b) -> p k b", k=KC)
            nc.vector.tensor_copy(send, o2_ps[:])
            nc.vector.tensor_copy(
                recvb[:, l, 0, :].rearrange("p (k b) -> p k b", k=KC), o2_ps[:])
            for d in range(1, 8):
                rd = [None] * 8
                rd[d] = (0, d)
                nc.gpsimd.remote_dma_broadcast(
                    recvb[:, l, d, :], sendb[:, l, :],
                    remote_sem=rsem, local_sem=lsem, rdests=rd)
            nc.gpsimd.trigger_dma(count=None)
            # gate writes zeros to slot 8 AND reads the send tile: the read
            # forces Tile to schedule it after the send-copy, so every core's
            # sends are in flight before its vector queue blocks on arrivals
            # (a dep-free memset could legally be scheduled first -> global
            # deadlock). The rsem wait itself is attached post-scheduling.
            gate = nc.vector.tensor_scalar_mul(recvb[:, l, 8, :], sendb[:, l, :], 0.0)
            post_waits.append((gate, rsem, 14 * (l + 1)))
            o2s = ab.tile([128, KC, B], F32, tag="o2s")
            nc.vector.tensor_reduce(
                o2s[:], recvb[:, l, :, :].rearrange("p s (k b) -> p k b s", k=KC),
                axis=AX.X, op=ALU.add)
        else:
            o2p = ab.tile([128, KC, B], F32, tag="o2p")
            nc.vector.tensor_copy(o2p[:], o2_ps[:])
            ar_i = dr.tile([128, KC * B], F32, tag="ari")
            ar_o = dr.tile([128, KC * B], F32, addr_space="Shared", tag="aro")
            nc.scalar.dma_start(ar_i[:].rearrange("p (k b) -> p k b", k=KC), o2p[:])
            nc.gpsimd.collective_compute(
                "AllReduce", ALU.add, replica_groups=groups,
                ins=[ar_i.opt()], outs=[ar_o.opt()],
            )
            o2s = ab.tile([128, KC, B], F32, tag="o2s")
            nc.scalar.dma_start(o2s[:], ar_o[:].rearrange("p (k b) -> p k b", k=KC))

        # --- residual + LN2
        y2 = ab.tile([128, KC, B], F32, tag="y2")
        nc.vector.tensor_tensor(y2[:], x1nT[:], o2s[:], op=ALU.add)
        xT, xT16 = layernorm(y2, ("xT", "xT16"))

        # --- token-major x for the NEXT layer's attention mix
        if l < L - 1:
            xtok_ps0 = ps.tile([B, NH], F16, tag="tm")
            xtok_ps1 = ps.tile([B, NH], F16, tag="tm")
            for j in range(KC):
                tgt = xtok_ps0 if j < 3 else xtok_ps1
                nc.tensor.transpose(tgt[:, 128 * (j % 3):128 * (j % 3 + 1)],
                                    xT16[:, j, :], ident128[:])
            x_tok16 = ab.tile([B, E], F16, tag="xtok")
            nc.scalar.activation(x_tok16[:, 0:NH], xtok_ps0[:], ACT_F.Copy)
            nc.scalar.activation(x_tok16[:, NH:E], xtok_ps1[:], ACT_F.Copy)

    # --- classifier: logits = x @ Wc
    wc_sb = wp.tile([128, KC, C], F16, tag="wcs")
    nc.sync.dma_start(wc_sb[:], h["wc"].ap().rearrange("(k p) n -> p k n", p=128))
    lg_ps = ps.tile([B, C], F32, tag="sm", bufs=1)
    for k in range(KC):
        nc.tensor.matmul(lg_ps[:], xT16[:, k, :], wc_sb[:, k, :],
                         start=(k == 0), stop=(k == KC - 1))
    lg_sb = ab.tile([B, C], F32, tag="lgs")
    nc.vector.tensor_copy(lg_sb[:], lg_ps[:])
    nc.sync.dma_start(h["out"].ap(), lg_sb[:])

    for p in reversed(ctxs):
        p.release()
    return post_waits


def build():
    if "k" in _CACHE:
        return _CACHE["k"]
    nc = bacc.Bacc("TRN2", target_bir_lowering=False, debug=False, num_devices=NC)
    h = _declare(nc)
    with tile.TileContext(nc) as tc:
        post_waits = _emit(tc, h)
    for inst, sem, thr in post_waits:
        inst._wait_ge(sem, thr)
    nc.compile()
    _CACHE["k"] = (nc, h)
    return nc, h


def make_in_maps(inputs):
    f32 = lambda a: np.asarray(a, dtype=np.float32)
    ids = np.asarray(inputs["input_ids"])[0]
    x0 = f32(inputs["tok_emb"])[ids] + f32(inputs["pos_emb"])[0][None, :]
    x0T = np.ascontiguousarray(x0.T.astype(np.float32))      # [768, 32] f32
    x0tok = np.ascontiguousarray(x0.astype(np.float16))      # [32, 768] f16

    shared = {"x0T": x0T, "x0tok": x0tok,
              "wc": np.ascontiguousarray(f32(inputs["Wc"]).astype(np.float16))}
    wqk, wvo, w1, w2 = [], [], [], []
    for l in range(L):
        qk = (f32(inputs["Wq"][l]) @ f32(inputs["Wk"][l]).T) * SCALE
        vo = f32(inputs["Wv"][l]) @ f32(inputs["Wo"][l])
        wqk.append(np.ascontiguousarray(qk.astype(np.float16)))
        wvo.append(np.ascontiguousarray(vo.astype(np.float16)))
        w1.append(f32(inputs["W1"][l]).astype(np.float16))
        w2.append(f32(inputs["W2"][l]).astype(np.float16))

    in_maps = []
    for c in range(NC):
        m = dict(shared)
        for l in range(L):
            m[f"wqk{l}"] = wqk[l]
            m[f"wvo{l}"] = wvo[l]
            m[f"w1{l}"] = np.ascontiguousarray(w1[l][:, FSH * c:FSH * (c + 1)])
            m[f"w2{l}"] = np.ascontiguousarray(w2[l][FSH * c:FSH * (c + 1), :])
        in_maps.append(m)
    return in_maps


def _nontrivial_bias(inputs):
    z = lambda *names: all(not np.any(np.asarray(inputs[n])) for n in names)
    use_bias = not z("bq", "bk", "bv", "bo", "bf1", "bf2", "bc")
    use_affine = not (
        z("beta1", "beta2")
        and np.all(np.asarray(inputs["g1"]) == 1.0)
        and np.all(np.asarray(inputs["g2"]) == 1.0)
    )
    return use_bias or use_affine


def _numpy_reference(inputs):
    # Exact CPU fallback (only taken if biases/affine are nontrivial).
    I = {k: np.asarray(v) for k, v in inputs.items()}
    x = I["tok_emb"][I["input_ids"]] + I["pos_emb"][np.arange(S)][:, None, :]
    x = x.astype(np.float32)
    scale = 1.0 / np.sqrt(E)

    def ln(t, g, b):
        mu = t.mean(-1, keepdims=True)
        var = t.var(-1, keepdims=True)
        return (t - mu) / np.sqrt(var + 1e-5) * g + b

    for l in range(L):
        Q = x @ I["Wq"][l] + I["bq"][l]
        K = x @ I["Wk"][l] + I["bk"][l]
        Vv = x @ I["Wv"][l] + I["bv"][l]
        sc = np.einsum('sbe,sce->sbc', Q, K) * scale
        sc = sc - sc.max(-1, keepdims=True)
        a = np.exp(sc)
        a /= a.sum(-1, keepdims=True)
        ao = np.einsum('sbc,sce->sbe', a, Vv) @ I["Wo"][l] + I["bo"][l]
        x = ln(x + ao, I["g1"][l], I["beta1"][l])
        hh = np.maximum(x @ I["W1"][l] + I["bf1"][l], 0.0) @ I["W2"][l] + I["bf2"][l]
        x = ln(x + hh, I["g2"][l], I["beta2"][l])
    return (x[0] @ I["Wc"] + I["bc"]).astype(np.float32)


def kernel(**inputs) -> np.ndarray:
    global LAST_RESULT
    if _nontrivial_bias(inputs):
        return _numpy_reference(inputs)
    nc, h = build()
    in_maps = make_in_maps(inputs)
    res = run_bass_kernel_spmd(nc, in_maps, core_ids=list(range(NC)))
    LAST_RESULT = res
    return np.asarray(res.results[0]["out"])


# revision 19
# speedup vs baseline: 26.3789x; 1.0817x over previous
"""Trainium2 Bass kernel for nn_BERTClassifier (batch-mixing attention BERT).

The reference output depends only on sequence position 0 (attention mixes the
batch within a position; every other op is position-local), so the real work
is a [32, 768] activation through 4 transformer layers — dominated by weight
streaming and inter-core latency, not FLOPs.

Design (v2):
- Everything is FEATURE-major ([128 partitions, 6 chunks, 32 tokens]): vector
  ops use all 128 lanes (4x faster than token-major), and every projection is
  a run of N=32 matmuls (lhsT = weight chunk, rhs = activation), which the PE
  pipelines at ~27ns/mm — the N=384 weight-moving form serializes on
  LDWEIGHTS at ~320ns/mm.
- Attention algebra folded on the host: scores = x (Wq Wk^T/sqrt(E)) x^T and
  ao Wo = (attn x) (Wv Wo). So the device sees TWO 768x768 matrices per layer
  (Wqk, Wvo) instead of four — half the attention HBM traffic and no separate
  Q/K/V/ao stages. Attention weights replicated on all 8 cores; FFN is 8-way
  tensor parallel (W1 col-shard, W2 row-shard) with ONE partial-sum exchange
  per layer.
- Exchange = all-to-all remote_dma_broadcast (SBUF->SBUF, XOR-delta slotting:
  delta d lands in recv slot d of core me^d), gated by a remote semaphore.
  No ncfw collectives at all -> no CC-stream entry barrier (37-85us in the
  baseline), no DRAM staging round trips (~15us/collective). The arrival
  waits are attached AFTER Tile scheduling (the scheduler's single-core sim
  cannot see remote increments and would deadlock).
- LayerNorm feature-major: PE ones-matmul column sums, tiny [1,32] stat ops,
  PE broadcast, fused normalize. Softmax skips max-subtraction (scores are
  O(1) by construction) and folds the 1/sum into the attn@x PSUM copy.
- Weights are cast fp32->fp16 on the HOST (halves HBM traffic; HWDGE loads).

Biases/affine are all zero in this problem instance; if a caller ever passes
nonzero ones, kernel() falls back to an exact numpy implementation.

Self-contained: shapes hardcoded, no sibling imports.
"""
import sys
import types

import numpy as np

# If BASS_TRACE is set but the axon NTFF hook module is absent, bass_utils
# would crash importing antenv.axon_hooks. Provide a null hook so tracing
# degrades to a warning instead. (test.py installs the real hook first.)
try:
    from antenv import axon_hooks as _ah  # noqa: F401
except ImportError:
    try:
        import antenv as _antenv
        _mod = types.ModuleType("antenv.axon_hooks")
        _mod.get_axon_ntff_profile_hook = lambda: None
        _mod.set_axon_ntff_profile_hook = lambda h: None
        _antenv.axon_hooks = _mod
        sys.modules["antenv.axon_hooks"] = _mod
    except Exception:
        pass

import concourse.bass as bass
import concourse.bacc as bacc
import concourse.mybir as mybir
import concourse.tile as tile
from concourse import masks
from concourse.bass_utils import run_bass_kernel_spmd

F32 = mybir.dt.float32
F16 = mybir.dt.float16
AX = mybir.AxisListType
ALU = mybir.AluOpType
ACT_F = mybir.ActivationFunctionType

V, E, F, L, S, B, C = 30522, 768, 3072, 4, 512, 32, 2
NC = 8             # cores
FSH = F // NC      # 384 ffn shard
KC = E // 128      # 6 contraction chunks of 128
KF = FSH // 128    # 3 contraction chunks for the W2 shard
NH = E // 2        # 384
SCALE = 1.0 / float(np.sqrt(E))
EPS = 1e-5

USE_REMOTE = False  # remote_dma exchange vs ncfw AllReduce (remote_dma
# delivery measured multi-ms in this axon-tunneled environment; ncfw
# collectives deliver in ~10us, so the exchange rides ncfw)

_CACHE = {}
LAST_RESULT = None  # BassKernelResults of the most recent run (for test.py)


def _declare(nc):
    h = {}
    h["x0T"] = nc.dram_tensor("x0T", [E, B], F32, kind="ExternalInput")
    h["x0tok"] = nc.dram_tensor("x0tok", [B, E], F16, kind="ExternalInput")
    for l in range(L):
        h[f"wqk{l}"] = nc.dram_tensor(f"wqk{l}", [E, E], F16, kind="ExternalInput")
        h[f"wvo{l}"] = nc.dram_tensor(f"wvo{l}", [E, E], F16, kind="ExternalInput")
        h[f"w1{l}"] = nc.dram_tensor(f"w1{l}", [E, FSH], F16, kind="ExternalInput")
        h[f"w2{l}"] = nc.dram_tensor(f"w2{l}", [FSH, E], F16, kind="ExternalInput")
    h["wc"] = nc.dram_tensor("wc", [E, C], F16, kind="ExternalInput")
    h["out"] = nc.dram_tensor("out", [B, C], F32, kind="ExternalOutput")
    return h


def _emit(tc, h):
    nc = tc.nc
    groups = [list(range(NC))]
    ctxs = []
    post_waits = []  # (BassInstruction, sem, threshold) applied post-scheduling

    def pool(*a, **k):
        p = tc.alloc_tile_pool(*a, **k)
        ctxs.append(p)
        return p

    const = pool(name="const", bufs=1)
    wp = pool(name="wts", bufs=2)
    ab = pool(name="act", bufs=2)
    ps = pool(name="ps", bufs=2, space="PSUM")

    ident32 = const.tile([B, B], F16)
    masks.make_identity(nc, ident32[:])
    ident128 = const.tile([128, 128], F16)
    masks.make_identity(nc, ident128[:])
    ones_col = const.tile([128, 1], F16)       # scaled by 1/E: mm gives mean
    nc.vector.memset(ones_col[:], 1.0 / E)
    ones_row = const.tile([1, 128], F16)
    nc.vector.memset(ones_row[:], 1.0)
    eps_sb = const.tile([1, 1], F32)
    nc.vector.memset(eps_sb[:], EPS)
    dum = const.tile([1, 1], F32)
    nc.vector.memset(dum[:], 1.0)

    if USE_REMOTE:
        rsem = nc.alloc_semaphore("rsem")
        lsem = nc.alloc_semaphore("lsem")
        # per-layer slots, never reused: no WAR hazards across layers.
        sendb = const.tile([128, L, 192], F16)
        recvb = const.tile([128, L, 9, 192], F16)
    else:
        dr = pool(name="dram", bufs=2, space="DRAM")

    # ---- embedding (position 0 only), both layouts from host
    xT = ab.tile([128, KC, B], F32, tag="xT0")
    nc.scalar.dma_start(xT[:], h["x0T"].ap().rearrange("(k p) b -> p k b", p=128))
    xT16 = ab.tile([128, KC, B], F16, tag="xT16_0")
    nc.vector.tensor_copy(xT16[:], xT[:])
    x_tok16 = ab.tile([B, E], F16, tag="xtok0")
    nc.scalar.dma_start(x_tok16[:], h["x0tok"].ap())

    def load_w(name, rows, cols, split=False):
        t = wp.tile([128, rows // 128, cols], F16, tag=name[:3])
        src = h[name].ap().rearrange("(k p) n -> p k n", p=128)
        if split:
            half = rows // 256
            nc.sync.dma_start(t[:, 0:half, :], src[:, 0:half, :])
            nc.sync.dma_start(t[:, half:, :], src[:, half:, :])
        else:
            nc.sync.dma_start(t[:], src)
        return t

    def layernorm(y_sb, y16, out_tags):
        # feature-major LN. Stats from the f16 copy (1-pass PE matmuls); the
        # normalize itself stays f32.
        sq = ab.tile([128, KC, B], F16, tag="sq")
        nc.vector.tensor_tensor(sq[:], y16[:], y16[:], op=ALU.mult)
        mu_ps = ps.tile([1, KC, B], F32, tag="st")
        nc.tensor.matmul(mu_ps[:], ones_col[:], y16[:], start=True, stop=True)
        m2_ps = ps.tile([1, KC, B], F32, tag="st")
        nc.tensor.matmul(m2_ps[:], ones_col[:], sq[:], start=True, stop=True)
        stat = ab.tile([1, 2, B], F32, tag="stat")
        nc.vector.tensor_reduce(
            stat[:, 0, :], mu_ps[:].rearrange("p k b -> p b k"), axis=AX.X, op=ALU.add)
        nc.vector.tensor_reduce(
            stat[:, 1, :], m2_ps[:].rearrange("p k b -> p b k"), axis=AX.X, op=ALU.add)
        musq = ab.tile([1, B], F32, tag="musq")
        nc.vector.tensor_tensor(musq[:], stat[:, 0, :], stat[:, 0, :], op=ALU.mult)
        var = ab.tile([1, B], F32, tag="var")
        nc.vector.tensor_tensor(var[:], stat[:, 1, :], musq[:], op=ALU.subtract)
        sd = ab.tile([1, B], F32, tag="sd")
        nc.scalar.activation(sd[:], var[:], ACT_F.Sqrt, bias=eps_sb[:])
        nc.vector.reciprocal(stat[:, 1, :], sd[:])
        stat16 = ab.tile([1, 2, B], F16, tag="stat16")
        nc.vector.tensor_copy(stat16[:], stat[:])
        bc_ps = ps.tile([128, 2, B], F32, tag="st")
        nc.tensor.matmul(bc_ps[:], ones_row[:], stat16[:], start=True, stop=True)
        mu_bb = bc_ps[:, 0:1, :].broadcast_to([128, KC, B])
        rs_bb = bc_ps[:, 1:2, :].broadcast_to([128, KC, B])
        tmp = ab.tile([128, KC, B], F32, tag="lntmp")
        nc.vector.tensor_tensor(tmp[:], y_sb[:], mu_bb, op=ALU.subtract)
        xnT = ab.tile([128, KC, B], F32, tag=out_tags[0])
        nc.vector.tensor_tensor(xnT[:], tmp[:], rs_bb, op=ALU.mult)
        xnT16 = ab.tile([128, KC, B], F16, tag=out_tags[1])
        nc.vector.tensor_copy(xnT16[:], xnT[:])
        return xnT, xnT16

    for l in range(L):
        wqk = load_w(f"wqk{l}", E, E, split=(l == 0))
        wvo = load_w(f"wvo{l}", E, E, split=(l == 0))
        w1 = load_w(f"w1{l}", E, FSH)
        w2 = load_w(f"w2{l}", FSH, E)

        # --- t^T = (x Wqk)^T  [128, KC, 32]
        tT_ps = ps.tile([128, KC, B], F32, tag="fm")
        for m in range(KC):
            for k in range(KC):
                nc.tensor.matmul(tT_ps[:, m, :], wqk[:, k, 128 * m:128 * (m + 1)],
                                 xT16[:, k, :], start=(k == 0), stop=(k == KC - 1))
        tT16 = ab.tile([128, KC, B], F16, tag="tT16")
        nc.vector.tensor_copy(tT16[:], tT_ps[:])

        # --- scores = t @ x^T [32, 32]; softmax without max-subtraction
        sc_ps = ps.tile([B, B], F32, tag="sm", bufs=1)
        for k in range(KC):
            nc.tensor.matmul(sc_ps[:], tT16[:, k, :], xT16[:, k, :],
                             start=(k == 0), stop=(k == KC - 1))
        attn16 = ab.tile([B, B], F16, tag="attn16")
        rsum = ab.tile([B, 1], F32, tag="rsum")
        nc.scalar.activation(attn16[:], sc_ps[:], ACT_F.Exp, accum_out=rsum[:])
        # dummy Sqrt right after Exp: forces the ACT table swap NOW, while
        # LN1's stats are still in flight (otherwise the 1.3us table load
        # sits fully exposed right before LN1's real Sqrt).
        dm = ab.tile([1, 1], F32, tag="dm")
        nc.scalar.activation(dm[:], dum[:], ACT_F.Sqrt, bias=eps_sb[:])
        rinv = ab.tile([B, 1], F32, tag="rinv")
        nc.vector.reciprocal(rinv[:], rsum[:])
        attnT16 = ab.tile([B, B], F16, tag="attnT16")
        nc.vector.transpose(attnT16[:], attn16[:])

        # --- xa = softmax(scores) @ x  (token mixing; 1/sum folded into copy)
        xa_ps0 = ps.tile([B, NH], F32, tag="tm")
        xa_ps1 = ps.tile([B, NH], F32, tag="tm")
        for n, xps in enumerate((xa_ps0, xa_ps1)):
            nc.tensor.matmul(xps[:], attnT16[:], x_tok16[:, NH * n:NH * (n + 1)],
                             start=True, stop=True)
        xa16 = ab.tile([B, E], F16, tag="xa16")
        nc.scalar.activation(xa16[:, 0:NH], xa_ps0[:], ACT_F.Copy, scale=rinv[:])
        nc.vector.tensor_scalar_mul(xa16[:, NH:E], xa_ps1[:], rinv[:])

        # --- xa^T via PE transposes, then o^T = (xa Wvo)^T
        xaT_ps = ps.tile([128, KC, B], F16, tag="fmT", bufs=1)
        for j in range(KC):
            nc.tensor.transpose(xaT_ps[:, j, :], xa16[:, 128 * j:128 * (j + 1)],
                                ident32[:])
        xaT16 = ab.tile([128, KC, B], F16, tag="xaT16")
        nc.vector.tensor_copy(xaT16[:], xaT_ps[:])
        oT_ps = ps.tile([128, KC, B], F32, tag="fm")
        for m in range(KC):
            for k in range(KC):
                nc.tensor.matmul(oT_ps[:, m, :], wvo[:, k, 128 * m:128 * (m + 1)],
                                 xaT16[:, k, :], start=(k == 0), stop=(k == KC - 1))

        # --- residual + LN1
        y1 = ab.tile([128, KC, B], F32, tag="y1")
        nc.vector.tensor_tensor(y1[:], xT[:], oT_ps[:], op=ALU.add)
        y116 = ab.tile([128, KC, B], F16, tag="y116")
        nc.scalar.activation(y116[:], y1[:], ACT_F.Copy)
        x1nT, x1nT16 = layernorm(y1, y116, ("x1nT", "x1nT16"))

        # --- FFN1 shard: h^T = relu(W1_c^T x1n^T) [128, KF, 32]
        hT_ps = ps.tile([128, KF, B], F32, tag="fm")
        for m in range(KF):
            for k in range(KC):
                nc.tensor.matmul(hT_ps[:, m, :], w1[:, k, 128 * m:128 * (m + 1)],
                                 x1nT16[:, k, :], start=(k == 0), stop=(k == KC - 1))
        hT16 = ab.tile([128, KF, B], F16, tag="hT16")
        nc.vector.tensor_scalar_max(hT16[:], hT_ps[:], 0.0)

        # --- FFN2 shard partial: o2p^T = W2_c^T h^T [128, KC, 32]
        o2_ps = ps.tile([128, KC, B], F32, tag="fm")
        for m in range(KC):
            for k in range(KF):
                nc.tensor.matmul(o2_ps[:, m, :], w2[:, k, 128 * m:128 * (m + 1)],
                                 hT16[:, k, :], start=(k == 0), stop=(k == KF - 1))

        # --- exchange: sum the 8 partial o2^T across cores
        if USE_REMOTE:
            send = sendb[:, l, :].rearrange("p (k b) -> p k b", k=KC)
            nc.vector.tensor_copy(send, o2_ps[:])
            nc.vector.tensor_copy(
                recvb[:, l, 0, :].rearrange("p (k b) -> p k b", k=KC), o2_ps[:])
            for d in range(1, 8):
                rd = [None] * 8
                rd[d] = (0, d)
                nc.gpsimd.remote_dma_broadcast(
                    recvb[:, l, d, :], sendb[:, l, :],
                    remote_sem=rsem, local_sem=lsem, rdests=rd)
            nc.gpsimd.trigger_dma(count=None)
            # gate writes zeros to slot 8 AND reads the send tile: the read
            # forces Tile to schedule it after the send-copy, so every core's
            # sends are in flight before its vector queue blocks on arrivals
            # (a dep-free memset could legally be scheduled first -> global
            # deadlock). The rsem wait itself is attached post-scheduling.
            gate = nc.vector.tensor_scalar_mul(recvb[:, l, 8, :], sendb[:, l, :], 0.0)
            post_waits.append((gate, rsem, 14 * (l + 1)))
            o2s = ab.tile([128, KC, B], F32, tag="o2s")
            nc.vector.tensor_reduce(
                o2s[:], recvb[:, l, :, :].rearrange("p s (k b) -> p k b s", k=KC),
                axis=AX.X, op=ALU.add)
        else:
            o2p = ab.tile([128, KC, B], F16, tag="o2p")
            nc.vector.tensor_copy(o2p[:], o2_ps[:])
            ar_i = dr.tile([128, KC * B], F16, tag="ari")
            ar_o = dr.tile([128, KC * B], F16, addr_space="Shared", tag="aro")
            nc.scalar.dma_start(ar_i[:].rearrange("p (k b) -> p k b", k=KC), o2p[:])
            nc.gpsimd.collective_compute(
                "AllReduce", ALU.add, replica_groups=groups,
                ins=[ar_i.opt()], outs=[ar_o.opt()],
            )
            # SWDGE cast-on-read DMA: f16 wire payload, f32 in SBUF
            o2s = ab.tile([128, KC, B], F32, tag="o2s")
            nc.gpsimd.dma_start(o2s[:], ar_o[:].rearrange("p (k b) -> p k b", k=KC))

        # --- residual + LN2
        y2 = ab.tile([128, KC, B], F32, tag="y2")
        nc.vector.tensor_tensor(y2[:], x1nT[:], o2s[:], op=ALU.add)
        y216 = ab.tile([128, KC, B], F16, tag="y216")
        nc.scalar.activation(y216[:], y2[:], ACT_F.Copy)
        xT, xT16 = layernorm(y2, y216, ("xT", "xT16"))

        # --- token-major x for the NEXT layer's attention mix
        if l < L - 1:
            xtok_ps0 = ps.tile([B, NH], F16, tag="tm")
            xtok_ps1 = ps.tile([B, NH], F16, tag="tm")
            for j in range(KC):
                tgt = xtok_ps0 if j < 3 else xtok_ps1
                nc.tensor.transpose(tgt[:, 128 * (j % 3):128 * (j % 3 + 1)],
                                    xT16[:, j, :], ident128[:])
            x_tok16 = ab.tile([B, E], F16, tag="xtok")
            nc.scalar.activation(x_tok16[:, 0:NH], xtok_ps0[:], ACT_F.Copy)
            nc.vector.tensor_copy(x_tok16[:, NH:E], xtok_ps1[:])

    # --- classifier: logits = x @ Wc
    wc_sb = wp.tile([128, KC, C], F16, tag="wcs")
    nc.sync.dma_start(wc_sb[:], h["wc"].ap().rearrange("(k p) n -> p k n", p=128))
    lg_ps = ps.tile([B, C], F32, tag="sm", bufs=1)
    for k in range(KC):
        nc.tensor.matmul(lg_ps[:], xT16[:, k, :], wc_sb[:, k, :],
                         start=(k == 0), stop=(k == KC - 1))
    lg_sb = ab.tile([B, C], F32, tag="lgs")
    nc.vector.tensor_copy(lg_sb[:], lg_ps[:])
    nc.sync.dma_start(h["out"].ap(), lg_sb[:])

    for p in reversed(ctxs):
        p.release()
    return post_waits


def build():
    if "k" in _CACHE:
        return _CACHE["k"]
    nc = bacc.Bacc("TRN2", target_bir_lowering=False, debug=False, num_devices=NC)
    h = _declare(nc)
    with tile.TileContext(nc) as tc:
        post_waits = _emit(tc, h)
    for inst, sem, thr in post_waits:
        inst._wait_ge(sem, thr)
    nc.compile()
    _CACHE["k"] = (nc, h)
    return nc, h


def make_in_maps(inputs):
    f32 = lambda a: np.asarray(a, dtype=np.float32)
    ids = np.asarray(inputs["input_ids"])[0]
    x0 = f32(inputs["tok_emb"])[ids] + f32(inputs["pos_emb"])[0][None, :]
    x0T = np.ascontiguousarray(x0.T.astype(np.float32))      # [768, 32] f32
    x0tok = np.ascontiguousarray(x0.astype(np.float16))      # [32, 768] f16

    shared = {"x0T": x0T, "x0tok": x0tok,
              "wc": np.ascontiguousarray(f32(inputs["Wc"]).astype(np.float16))}
    wqk, wvo, w1, w2 = [], [], [], []
    for l in range(L):
        qk = (f32(inputs["Wq"][l]) @ f32(inputs["Wk"][l]).T) * SCALE
        vo = f32(inputs["Wv"][l]) @ f32(inputs["Wo"][l])
        wqk.append(np.ascontiguousarray(qk.astype(np.float16)))
        wvo.append(np.ascontiguousarray(vo.astype(np.float16)))
        w1.append(f32(inputs["W1"][l]).astype(np.float16))
        w2.append(f32(inputs["W2"][l]).astype(np.float16))

    in_maps = []
    for c in range(NC):
        m = dict(shared)
        for l in range(L):
            m[f"wqk{l}"] = wqk[l]
            m[f"wvo{l}"] = wvo[l]
            m[f"w1{l}"] = np.ascontiguousarray(w1[l][:, FSH * c:FSH * (c + 1)])
            m[f"w2{l}"] = np.ascontiguousarray(w2[l][FSH * c:FSH * (c + 1), :])
        in_maps.append(m)
    return in_maps


def _nontrivial_bias(inputs):
    z = lambda *names: all(not np.any(np.asarray(inputs[n])) for n in names)
    use_bias = not z("bq", "bk", "bv", "bo", "bf1", "bf2", "bc")
    use_affine = not (
        z("beta1", "beta2")
        and np.all(np.asarray(inputs["g1"]) == 1.0)
        and np.all(np.asarray(inputs["g2"]) == 1.0)
    )
    return use_bias or use_affine


def _numpy_reference(inputs):
    # Exact CPU fallback (only taken if biases/affine are nontrivial).
    I = {k: np.asarray(v) for k, v in inputs.items()}
    x = I["tok_emb"][I["input_ids"]] + I["pos_emb"][np.arange(S)][:, None, :]
    x = x.astype(np.float32)
    scale = 1.0 / np.sqrt(E)

    def ln(t, g, b):
        mu = t.mean(-1, keepdims=True)
        var = t.var(-1, keepdims=True)
        return (t - mu) / np.sqrt(var + 1e-5) * g + b

    for l in range(L):
        Q = x @ I["Wq"][l] + I["bq"][l]
        K = x @ I["Wk"][l] + I["bk"][l]
        Vv = x @ I["Wv"][l] + I["bv"][l]
        sc = np.einsum('sbe,sce->sbc', Q, K) * scale
        sc = sc - sc.max(-1, keepdims=True)
        a = np.exp(sc)
        a /= a.sum(-1, keepdims=True)
        ao = np.einsum('sbc,sce->sbe', a, Vv) @ I["Wo"][l] + I["bo"][l]
        x = ln(x + ao, I["g1"][l], I["beta1"][l])
        hh = np.maximum(x @ I["W1"][l] + I["bf1"][l], 0.0) @ I["W2"][l] + I["bf2"][l]
        x = ln(x + hh, I["g2"][l], I["beta2"][l])
    return (x[0] @ I["Wc"] + I["bc"]).astype(np.float32)


def kernel(**inputs) -> np.ndarray:
    global LAST_RESULT
    if _nontrivial_bias(inputs):
        return _numpy_reference(inputs)
    nc, h = build()
    in_maps = make_in_maps(inputs)
    res = run_bass_kernel_spmd(nc, in_maps, core_ids=list(range(NC)))
    LAST_RESULT = res
    return np.asarray(res.results[0]["out"])
